# revision 36
# baseline (speedup 1.0000x reference)
"""Trainium2 Bass kernel for nn_CrossAttentionLayer (B=2,S=2048,H=768,NH=12).

Sharding: 8 cores = 2 batches x 4 head-groups (3 heads each, 192 cols).
Each core computes LN(hidden), q/k/v projections for its head slice,
attention (softmax without max-subtraction; denominator via ones-column),
and a partial output projection  attn_local @ Wo[rows_slice] * gate * dyn.
Host sums the 4 partials per batch (tensor-parallel unshard) — bias terms
are carried on the g==0 core via an extra contraction row.

All matmuls run as float32r (fp32 data, single-pass PE mode).
"""

import os
import sys
from contextlib import ExitStack

import numpy as np

sys.path.insert(0, "/opt/trn_rl_repo")

import concourse.bass as bass
import concourse.bacc as bacc
import concourse.tile as tile
from concourse import mybir
from concourse.tile import TileContext

B, S, H, NH = 2, 2048, 768, 12
HD = H // NH            # 64
NG = 4                  # head groups
HL = H // NG            # 192 local cols (3 heads)
NHL = NH // NG          # 3 local heads
MEM_W = 0.5
LN_EPS = 1e-5

F32 = mybir.dt.float32
F32R = mybir.dt.float32r

_CACHED = {}


def build_bass():
    nc = bacc.Bacc()

    hid = nc.declare_dram_parameter("hid", [S, H], F32, isOutput=False)
    crs = nc.declare_dram_parameter("crs", [S, H], F32, isOutput=False)
    m0 = nc.declare_dram_parameter("m0", [S, HL], F32, isOutput=False)
    m1 = nc.declare_dram_parameter("m1", [S, HL], F32, isOutput=False)
    wq = nc.declare_dram_parameter("wq", [H, HL], F32R, isOutput=False)
    wk = nc.declare_dram_parameter("wk", [H, HL], F32R, isOutput=False)
    wv = nc.declare_dram_parameter("wv", [H, HL], F32R, isOutput=False)
    wo = nc.declare_dram_parameter("wo", [HL + 1, H], F32R, isOutput=False)
    bqv = nc.declare_dram_parameter("bqv", [128, 2], F32, isOutput=False)  # packed bq_eff
    bvv = nc.declare_dram_parameter("bvv", [64, 3], F32, isOutput=False)   # bv per head
    dynv = nc.declare_dram_parameter("dynv", [S], F32, isOutput=False)
    ident = nc.declare_dram_parameter("ident", [128, 128], F32, isOutput=False)
    out = nc.declare_dram_parameter("out", [S, H], F32, isOutput=True)

    NT = S // 128           # 16 s/t tiles
    NC_ = 4                 # 512-wide chunks
    MT = [(0, 128), (128, 64)]  # m-tiles of the 192 local cols

    with TileContext(nc) as tc, ExitStack() as ctx:
        # ---- persistent pools ----
        singles = ctx.enter_context(tc.tile_pool(name="singles", bufs=1))
        qkpool = ctx.enter_context(tc.tile_pool(name="qk", bufs=1))
        vpool = ctx.enter_context(tc.tile_pool(name="vsb", bufs=1))
        catpool = ctx.enter_context(tc.tile_pool(name="cat", bufs=1))

        id_sb = singles.tile([128, 128], F32)
        nc.sync.dma_start(out=id_sb, in_=ident[:])
        one_sb = singles.tile([128, 1], F32)
        nc.vector.memset(one_sb, 1.0)
        ones_sb = singles.tile([1, 64], F32R)
        nc.vector.tensor_copy(ones_sb, one_sb[0:1, 0:1].to_broadcast((1, 64)))
        eps_sb = singles.tile([128, 1], F32)
        nc.vector.memset(eps_sb, LN_EPS)
        bq_sb = singles.tile([128, 2], F32)
        nc.sync.dma_start(out=bq_sb, in_=bqv[:])
        dyn_sb = singles.tile([128, NT], F32)
        nc.sync.dma_start(out=dyn_sb, in_=dynv[:].rearrange("(c p) -> p c", p=128))

        wo_sb = [singles.tile([128, H], F32R, name="wo0", tag="wo0"),
                 singles.tile([65, H], F32R, name="wo1", tag="wo1")]
        nc.sync.dma_start(out=wo_sb[0], in_=wo[0:128, :])
        nc.sync.dma_start(out=wo_sb[1], in_=wo[128:193, :])

        # q/k transposed projections [m, s]; m-tiles: [128] + [64]
        qT = [qkpool.tile([128, S], F32R, name="qT0", tag="qT0"), qkpool.tile([64, S], F32R, name="qT1", tag="qT1")]
        kT = [qkpool.tile([128, S], F32R, name="kT0", tag="kT0"), qkpool.tile([64, S], F32R, name="kT1", tag="kT1")]
        # v in [t, m] layout with interleaved ones columns: per head 65 cols
        v_sb = [vpool.tile([128, 3 * 65], F32R, name=f"v{t}", tag=f"v{t}") for t in range(NT)]
        for t in range(NT):
            for h in range(3):
                nc.gpsimd.tensor_copy(out=v_sb[t][:, 65 * h + 64:65 * h + 65], in_=one_sb)
        # attention output accumulators [m, s] (+ ones row for bias)
        cat0 = catpool.tile([128, S], F32R, tag="cat0")
        cat1 = catpool.tile([65, S], F32R, tag="cat1")
        nc.gpsimd.tensor_copy(out=cat1[64:65, :], in_=one_sb[0:1, 0:1].to_broadcast((1, S)))

        # ---- phase 1+2: LN, transposes, projections ----
        with tc.tile_pool(name="bigT", bufs=1) as bigT, \
             tc.tile_pool(name="rows768", bufs=2) as rows768, \
             tc.tile_pool(name="stats", bufs=3) as stats, \
             tc.tile_pool(name="m01", bufs=1) as m01, \
             tc.tile_pool(name="vT", bufs=1) as vTp, \
             tc.tile_pool(name="wpool", bufs=1) as wpool, \
             tc.tile_pool(name="pps", bufs=3, space="PSUM") as pps, \
             tc.tile_pool(name="ppt", bufs=4, space="PSUM") as ppt:

            wq_sb = [wpool.tile([128, HL], F32R, name=f"wq{j}", tag=f"wq{j}") for j in range(6)]
            wk_sb = [wpool.tile([128, HL], F32R, name=f"wk{j}", tag=f"wk{j}") for j in range(6)]
            wv_sb = [wpool.tile([128, HL], F32R, name=f"wv{j}", tag=f"wv{j}") for j in range(6)]
            for j in range(6):
                nc.sync.dma_start(out=wq_sb[j], in_=wq[j * 128:(j + 1) * 128, :])
                nc.gpsimd.dma_start(out=wk_sb[j], in_=wk[j * 128:(j + 1) * 128, :])
                nc.gpsimd.dma_start(out=wv_sb[j], in_=wv[j * 128:(j + 1) * 128, :])

            # --- hidden -> LN -> xT ---
            xT = [bigT.tile([128, S], F32R, name=f"bigT{j}", tag=f"bigT{j}") for j in range(6)]
            for c in range(NC_):
                xh = []
                for k in range(4):
                    i = 4 * c + k
                    ht = rows768.tile([128, H], F32, tag=f"r768_{k}")
                    nc.sync.dma_start(out=ht, in_=hid[i * 128:(i + 1) * 128, :])
                    st = stats.tile([128, 3, 6], F32, tag=f"st{k}")
                    for sg in range(3):
                        nc.vector.bn_stats(out=st[:, sg, :], in_=ht[:, sg * 256:(sg + 1) * 256])
                    mv = stats.tile([128, 2], F32, tag=f"mv{k}")
                    nc.vector.bn_aggr(out=mv, in_=st)
                    rstd = stats.tile([128, 1], F32, tag=f"rs{k}")
                    nc.scalar.activation(out=rstd, in_=mv[:, 1:2],
                                         func=mybir.ActivationFunctionType.Sqrt,
                                         bias=eps_sb, scale=1.0)
                    nc.vector.reciprocal(out=rstd, in_=rstd)
                    nc.vector.tensor_scalar(out=ht, in0=ht, scalar1=mv[:, 0:1],
                                            scalar2=rstd,
                                            op0=mybir.AluOpType.subtract,
                                            op1=mybir.AluOpType.mult)
                    xh.append(ht)
                for j in range(6):
                    pt = ppt.tile([128, 512], F32, tag="pt")
                    for k in range(4):
                        nc.tensor.transpose(pt[:, k * 128:(k + 1) * 128],
                                            xh[k][:, j * 128:(j + 1) * 128], id_sb)
                    nc.scalar.copy(xT[j][:, c * 512:(c + 1) * 512], pt)

            # --- qT projection (+bq) ---
            for mi, (m0_, msz) in enumerate(MT):
                for n in range(NC_):
                    ps = pps.tile([128, 512], F32, tag="proj")
                    for j in range(6):
                        nc.tensor.matmul(ps[:msz], wq_sb[j][:, m0_:m0_ + msz],
                                         xT[j][:, n * 512:(n + 1) * 512],
                                         start=(j == 0), stop=(j == 5))
                    nc.vector.tensor_scalar(out=qT[mi][:, n * 512:(n + 1) * 512],
                                            in0=ps[:msz], scalar1=bq_sb[:msz, mi:mi + 1],
                                            scalar2=None, op0=mybir.AluOpType.add)

            # --- cross -> crossT (reuses bigT slots after qT reads retire) ---
            cT = [bigT.tile([128, S], F32R, name=f"bigT{j}", tag=f"bigT{j}") for j in range(6)]
            for c in range(NC_):
                xh = []
                for k in range(4):
                    i = 4 * c + k
                    ht = rows768.tile([128, H], F32, tag=f"r768_{k}")
                    nc.gpsimd.dma_start(out=ht, in_=crs[i * 128:(i + 1) * 128, :])
                    xh.append(ht)
                for j in range(6):
                    pt = ppt.tile([128, 512], F32, tag="pt")
                    for k in range(4):
                        nc.tensor.transpose(pt[:, k * 128:(k + 1) * 128],
                                            xh[k][:, j * 128:(j + 1) * 128], id_sb)
                    nc.scalar.copy(cT[j][:, c * 512:(c + 1) * 512], pt)

            # --- m0 tiles (for kT add) ---
            m0_sb = [m01.tile([128, HL], F32, name=f"m{t}", tag=f"m{t}") for t in range(NT)]
            for t in range(NT):
                nc.gpsimd.dma_start(out=m0_sb[t], in_=m0[t * 128:(t + 1) * 128, :])

            # --- kT projection + mem0^T ---
            for mi, (m0_, msz) in enumerate(MT):
                for n in range(NC_):
                    ps = pps.tile([128, 512], F32, tag="proj")
                    for j in range(6):
                        nc.tensor.matmul(ps[:msz], wk_sb[j][:, m0_:m0_ + msz],
                                         cT[j][:, n * 512:(n + 1) * 512],
                                         start=(j == 0), stop=(j == 5))
                    pm = ppt.tile([128, 512], F32, tag="pt")
                    for k in range(4):
                        nc.tensor.transpose(pm[:msz, k * 128:(k + 1) * 128],
                                            m0_sb[4 * n + k][:, m0_:m0_ + msz], id_sb)
                    kdst = kT[mi][:, n * 512:(n + 1) * 512]
                    nc.scalar.copy(kdst, ps[:msz])
                    nc.vector.tensor_tensor(kdst, kdst, pm[:msz],
                                            mybir.AluOpType.add)

            # --- vT projection, then transpose into v_sb [t, m] + mem1 ---
            vT = [vTp.tile([128, S], F32, name="vT0", tag="vT0"), vTp.tile([64, S], F32, name="vT1", tag="vT1")]
            for mi, (m0_, msz) in enumerate(MT):
                for n in range(NC_):
                    ps = pps.tile([128, 512], F32, tag="proj")
                    for j in range(6):
                        nc.tensor.matmul(ps[:msz], wv_sb[j][:, m0_:m0_ + msz],
                                         cT[j][:, n * 512:(n + 1) * 512],
                                         start=(j == 0), stop=(j == 5))
                    nc.scalar.copy(vT[mi][:, n * 512:(n + 1) * 512], ps[:msz])

            m1_sb = [m01.tile([128, HL], F32, name=f"m{t}", tag=f"m{t}") for t in range(NT)]
            for t in range(NT):
                nc.gpsimd.dma_start(out=m1_sb[t], in_=m1[t * 128:(t + 1) * 128, :])
            for t in range(NT):
                pv = ppt.tile([128, 512], F32, tag="pt")
                nc.tensor.transpose(pv[:, 0:128], vT[0][:, t * 128:(t + 1) * 128], id_sb)
                nc.tensor.transpose(pv[:, 128:192],
                                    vT[1][:, t * 128:(t + 1) * 128], id_sb[:64, :64])
                for h in range(3):
                    nc.vector.tensor_tensor(v_sb[t][:, 65 * h:65 * h + 64],
                                            pv[:, 64 * h:64 * h + 64],
                                            m1_sb[t][:, 64 * h:64 * h + 64],
                                            mybir.AluOpType.add)

        # ---- phase 3+4: attention fused with per-chunk output projection ----
        CW = 1024
        with tc.tile_pool(name="expP", bufs=2) as expP, \
             tc.tile_pool(name="srec", bufs=1) as srec, \
             tc.tile_pool(name="osb", bufs=2) as osb, \
             tc.tile_pool(name="sps", bufs=2, space="PSUM") as sps, \
             tc.tile_pool(name="ops", bufs=1, space="PSUM") as ops, \
             tc.tile_pool(name="wps", bufs=2, space="PSUM") as wps:
            for n in range(S // CW):
                for h in range(3):
                    kk = kT[0][64 * h:64 * h + 64, :] if h < 2 else kT[1][0:64, :]
                    qq = qT[0][64 * h:64 * h + 64, :] if h < 2 else qT[1][0:64, :]
                    dst_all = cat0 if h < 2 else cat1
                    doff = 64 * h if h < 2 else 0
                    ets = []
                    for t in range(NT):
                        sp = sps.tile([128, CW], F32, tag="sc")
                        for v2 in range(CW // 512):
                            nc.tensor.matmul(sp[:, v2 * 512:(v2 + 1) * 512],
                                             kk[:, t * 128:(t + 1) * 128],
                                             qq[:, n * CW + v2 * 512:n * CW + (v2 + 1) * 512],
                                             start=True, stop=True)
                        et = expP.tile([128, CW], F32R, tag=f"e{t}", bufs=(2 if t < 14 else 1))
                        nc.scalar.activation(out=et, in_=sp,
                                             func=mybir.ActivationFunctionType.Exp,
                                             scale=float(1.0 / np.sqrt(HD)))
                        ets.append(et)
                    op_ = ops.tile([65, CW], F32, tag="ov")
                    for t in range(NT):
                        for v2 in range(CW // 512):
                            nc.tensor.matmul(op_[:, v2 * 512:(v2 + 1) * 512],
                                             v_sb[t][:, 65 * h:65 * h + 65],
                                             ets[t][:, v2 * 512:(v2 + 1) * 512],
                                             start=(t == 0), stop=(t == NT - 1))
                    rec32 = srec.tile([1, CW], F32, tag="rec32")
                    nc.vector.reciprocal(out=rec32, in_=op_[64:65])
                    rec = srec.tile([1, CW], F32R, tag="rec")
                    nc.vector.tensor_copy(rec, rec32)
                    cp = srec.tile([64, CW], F32, tag="cp")
                    nc.vector.tensor_copy(cp, op_[0:64])
                    # broadcast 1/denom into the (now spare) attnv PSUM rows
                    for v2 in range(CW // 512):
                        nc.tensor.matmul(op_[0:64, v2 * 512:(v2 + 1) * 512], ones_sb,
                                         rec[:, v2 * 512:(v2 + 1) * 512],
                                         start=True, stop=True)
                    dst = dst_all[doff:doff + 64, n * CW:(n + 1) * CW]
                    nc.vector.tensor_tensor(dst, cp, op_[0:64], mybir.AluOpType.mult)
                # output projection for the s-rows finished by this chunk
                for st_ in range(n * (CW // 128), (n + 1) * (CW // 128)):
                    ot = osb.tile([128, H], F32, tag="ot")
                    for half, n0 in enumerate((0, 384)):
                        wp = wps.tile([128, 384], F32, tag="wo")
                        nc.tensor.matmul(wp, cat0[:, st_ * 128:(st_ + 1) * 128],
                                         wo_sb[0][:, n0:n0 + 384],
                                         start=True, stop=False)
                        nc.tensor.matmul(wp, cat1[:, st_ * 128:(st_ + 1) * 128],
                                         wo_sb[1][:, n0:n0 + 384],
                                         start=False, stop=True)
                        nc.vector.tensor_scalar(out=ot[:, n0:n0 + 384], in0=wp,
                                                scalar1=dyn_sb[:, st_:st_ + 1],
                                                scalar2=None,
                                                op0=mybir.AluOpType.mult)
                    nc.sync.dma_start(out=out[st_ * 128:(st_ + 1) * 128, :], in_=ot)

    nc.compile()
    return nc


def make_in_maps(inputs):
    hs = np.asarray(inputs["hidden_states"], np.float32)
    cs = np.asarray(inputs["cross_states"], np.float32)
    mem = np.asarray(inputs["memory_tensors"], np.float32)
    dyn = np.asarray(inputs["dynamic_factor"], np.float32)
    Wq = np.asarray(inputs["Wq"], np.float32)
    Wk = np.asarray(inputs["Wk"], np.float32)
    Wv = np.asarray(inputs["Wv"], np.float32)
    Wo = np.asarray(inputs["Wo"], np.float32)
    bq = np.asarray(inputs["bq"], np.float32)
    bv = np.asarray(inputs["bv"], np.float32)
    bo = np.asarray(inputs["bo"], np.float32)
    gate = float(np.asarray(inputs["gate"]).reshape(-1)[0])
    gate_bias = float(np.asarray(inputs["gate_bias"]).reshape(-1)[0])
    ln_g = np.asarray(inputs["ln_g"], np.float32)
    ln_b = np.asarray(inputs["ln_b"], np.float32)

    ident = np.eye(128, dtype=np.float32)
    in_maps = []
    for core in range(8):
        b, g = divmod(core, NG)
        cols = slice(g * HL, (g + 1) * HL)
        wq_eff = (ln_g[:, None] * Wq[:, cols]).astype(np.float32)
        bq_eff = (bq[cols] + ln_b @ Wq[:, cols]).astype(np.float32)
        bq_pack = np.zeros((128, 2), np.float32)
        bq_pack[:, 0] = bq_eff[0:128]
        bq_pack[:64, 1] = bq_eff[128:192]
        bv_pack = np.asarray(bv[cols].reshape(3, 64).T, np.float32)  # [64,3]
        wo_ext = np.zeros((HL + 1, H), np.float32)
        wo_ext[:HL] = Wo[cols, :] * gate
        wo_ext[HL] = bv[cols] @ (Wo[cols, :] * gate)
        if g == 0:
            wo_ext[HL] += bo * gate + gate_bias
        in_maps.append({
            "hid": np.ascontiguousarray(hs[b]),
            "crs": np.ascontiguousarray(cs[b]),
            "m0": np.ascontiguousarray(mem[0, b][:, cols] * MEM_W),
            "m1": np.ascontiguousarray(mem[1, b][:, cols] * MEM_W),
            "wq": wq_eff,
            "wk": np.ascontiguousarray(Wk[:, cols]),
            "wv": np.ascontiguousarray(Wv[:, cols]),
            "wo": wo_ext,
            "bqv": bq_pack,
            "bvv": bv_pack,
            "dynv": np.ascontiguousarray(dyn[b, :, 0]),
            "ident": ident,
        })
    return in_maps


def kernel(**inputs):
    mask = np.asarray(inputs["attention_mask"])
    if not np.all(mask != 0):
        raise NotImplementedError("kernel specialized for all-ones attention_mask")

    if "nc" not in _CACHED:
        _CACHED["nc"] = build_bass()
    nc = _CACHED["nc"]

    from concourse.bass_utils import run_bass_kernel_spmd
    in_maps = make_in_maps(inputs)
    trace = bool(int(os.environ.get("KERNEL_TRACE", "0")))
    r = run_bass_kernel_spmd(nc, in_maps, list(range(8)), trace=trace)
    _CACHED["exec_time_ns"] = r.exec_time_ns
    _CACHED["profile_json"] = r.profile_json
    _CACHED["trace"] = r.instructions_and_trace
    res = r.results

    out = np.zeros((B, S, H), np.float32)
    for core in range(8):
        b = core // NG
        out[b] += res[core]["out"]
    return out
